# revision 2
# baseline (speedup 1.0000x reference)
"""Bezier-stroke rasterizer (AIR/Guide-style) as a Trainium2 Bass/Tile kernel.

v2 redesign vs the selector-matrix baseline:
- d2 = (c-g)^2 is computed DIRECTLY by the PE as 64 tiny per-pair matmuls
  [7,T]x[7,28]: contraction rows are free (matmul cost ~ output cols), the
  rhs is ONE shared [7,28] constant (G hi/lo, ones, g^2 hi/lo) and the lhs
  is per-pair fp16 hi/lo c-data -> the 118KB selector matrix is gone; the
  critical input DMA drops to ~140KB.
- The gaussian is Exp(-inv*d2 + 0.5*ln w_t): Exp lives in act table set 0
  together with Tanh/Copy, so the WHOLE kernel uses one table set -> the
  1283ns mid-kernel table reload and its dummy-tanh hoist are gone.  The
  trapezoid weight folds into Exp's per-partition bias (0.5*ln w on both
  coords) -> the DVE weight-multiply is gone too.
- T=128 curve samples (quadrature error ~1e-8 for worst-case stroke
  lengths); trapezoid weights fold into the host-side c^2 rows.
- Output leaves via a Pool SWDGE kv_writeback whose descriptors are
  prepped during the input-DMA wait and merely TRIGGERED after the final
  activation: the 625ns HWDGE + 650ns DGE delay vanish from the tail.

Sharding: pure data parallel, 8 batches per core across 8 NeuronCores.
"""

import sys
import numpy as np
from math import comb, tanh

sys.path.insert(0, "/opt/trn_rl_repo")

from concourse import bass, bacc, tile, mybir  # noqa: E402
from concourse.bass_utils import run_bass_kernel_spmd  # noqa: E402

BS, K, PTS, RES = 64, 4, 5, 28
STEPS_REF = 500
T = 128                     # curve samples on device
NCORES = 8
BL = BS // NCORES           # local batches per core = 8
NPAIR = BL * K              # (batch, stroke) pairs per core = 32
NA = 16                     # pairs in PSUM bank0 chunk (= batches g 0..3)
NB = NPAIR - NA             # pairs in bank1 chunk (= batches g 4..7)
RPP = 7                     # lhs rows per pair
NH = 2                      # stroke column-halves (j//2)
SP = 64                     # stroke partition extent (2 x 32-padded)
EPS = 1e-6
F32 = mybir.dt.float32
F16 = mybir.dt.float16
BF16 = mybir.dt.bfloat16
AF = mybir.ActivationFunctionType
ALU = mybir.AluOpType

# lhs layout: two tensors on partitions 0-6 (matmul base-partition rule):
#   lhsy [7, LYC]: pair p's y c-rows at cols p*T; gmat at col 32*T
#   lhsx [7, 32*T]: pair p's x c-rows at cols p*T
# The Exp bias 0.5*ln(w) has only two distinct values (trapezoid weights) ->
# built on-device with 3 memsets, no DMA.  EPS/zp rides in 4 extra bf16
# columns of deltah (bitcast to f32).
C_G = NPAIR * T
LYC = C_G + RES
LXC = NPAIR * T
DHC = NH * BL * RES + 4     # deltah cols + 4 bf16 cols holding ezi f32


def _pair_col(p):
    """column base of pair p in the [*, 1024] d2/E tiles"""
    return 28 * p if p < NA else 512 + 28 * (p - NA)


def _build_program(sigma, slope_strk, slope):
    inv = 1.0 / (2.0 * sigma * sigma)
    post1 = 1.0 / tanh(slope_strk)

    nc = bacc.Bacc(None, target_bir_lowering=False)

    lhsy_d = nc.dram_tensor("lhsy", [RPP, LYC], F16, kind="ExternalInput")
    lhsx_d = nc.dram_tensor("lhsx", [RPP, LXC], F16, kind="ExternalInput")
    dl_d = nc.dram_tensor("deltah", [NH * BL, DHC], BF16,
                          kind="ExternalInput")
    ks_d = nc.dram_tensor("ksum", [SP, RES], BF16, kind="ExternalInput")
    out_d = nc.dram_tensor("out", [BL, 128, 1, RES], BF16,
                           kind="ExternalOutput")

    with tile.TileContext(nc) as tc:
        with (
            tc.tile_pool(name="const", bufs=1) as cpool,
            tc.tile_pool(name="work", bufs=1) as wpool,
            tc.tile_pool(name="dsq", bufs=2, space="PSUM") as dpool,
            tc.tile_pool(name="sp", bufs=1, space="PSUM") as spool,
        ):
            # ---- input DMAs (SP queue: lhsy critical-first) ----
            lhsy = cpool.tile([RPP, LYC], F16)
            nc.sync.dma_start(lhsy[:], lhsy_d[:])
            lhsx = cpool.tile([RPP, LXC], F16)
            nc.sync.dma_start(lhsx[:], lhsx_d[:])
            deltaHz = cpool.tile([NH * BL, DHC], BF16)
            nc.sync.dma_start(deltaHz[:], dl_d[:])
            ksum = cpool.tile([SP, RES], BF16)
            nc.sync.dma_start(ksum[:], ks_d[:])

            gmat = lhsy[0:RPP, C_G : C_G + RES]
            deltaH = deltaHz[:, 0 : NH * BL * RES]
            ezi = deltaHz[:, NH * BL * RES : DHC].bitcast(F32)  # [16, 2] f32

            # ---- act-table hoist ----
            scratch = wpool.tile([128, 512], BF16)
            nc.vector.memset(scratch[:], 0.0)
            zbias = wpool.tile([128, 1], F32)
            nc.vector.memset(zbias[:], 0.0)
            expsc = wpool.tile([1, 8], F32)
            # dummy Exp: places the single set-0 table load in the DMA wait
            nc.scalar.activation(expsc[:], scratch[0:1, 0:8], AF.Exp,
                                 bias=zbias[0:1, :])

            # output kv_writeback: desc-gen on the idle Pool engine NOW --
            # `at` has no producer yet so the prep carries no data wait; the
            # trigger at the end is repointed (post-compile) at the final
            # activation's engine tick.
            at = wpool.tile([128, 1, BL, RES], BF16)
            kidx = wpool.tile([128, BL], mybir.dt.int32)
            nc.gpsimd.memset(kidx[:], 0)
            ksem = nc.alloc_semaphore("out_dma")
            osem = nc.alloc_semaphore("out_ready")
            nc.gpsimd.kv_writeback(out_d[:], at[:], kidx[:],
                                   prepare_only=True, sem=ksem, queue_num=0)

            # ---- d2 matmuls: 64 x [7,T]x[7,28], then Exp per coord ----
            # d2 tile [T, 1024]: pairs 0:16 (batches g<4) in bank0 cols
            # 0:448, pairs 16:32 in bank1 cols 512:960.  The acts read a
            # strided [T, 2, 448] view so the pad columns cost nothing.
            # ln(w) is folded into the host-side c^2 rows -> bias is zero.
            def _emit_mms(lt, d2, lo, hi):
                for p in range(lo, hi):
                    cb = _pair_col(p)
                    nc.tensor.matmul(
                        d2[:, cb : cb + RES],
                        lt[0:RPP, p * T : (p + 1) * T],
                        gmat,
                        start=True, stop=True,
                    )

            def _v(tile_):
                return tile_[:].rearrange(
                    "p (a b) -> p a b", a=2, b=512)[:, :, 0:448]

            d2y = dpool.tile([T, 1024], F32, tag="d2")
            _emit_mms(lhsy, d2y, 0, NPAIR)
            Ey = wpool.tile([T, 1024], F16, name="Ey")
            nc.scalar.activation(_v(Ey), _v(d2y),
                                 AF.Exp, bias=zbias[0:T, :], scale=-float(inv))
            d2x = dpool.tile([T, 1024], F32, tag="d2")
            Ex = wpool.tile([T, 1024], F16, name="Ex")
            _emit_mms(lhsx, d2x, 0, NPAIR)
            nc.scalar.activation(_v(Ex), _v(d2x),
                                 AF.Exp, bias=zbias[0:T, :], scale=-float(inv))

            # ---- stroke maps: 32 garbage-free [28,28] matmuls ----
            # stroke (g, j) = pair p = 4g+j -> partition base 32*(j%2),
            # column (j//2, g, :).  Chunk g<4 first: its strokes only need
            # the first x-act, and its row-max runs while chunk B strokes
            # are still in flight.
            S_all = spool.tile([SP, NH, BL, RES], F32, tag="S")
            nc.vector.memset(S_all[:], 0.0)
            for g in range(BL):
                for j in range(K):
                    p = 4 * g + j
                    cb = _pair_col(p)
                    nc.tensor.matmul(
                        S_all[32 * (j % 2) : 32 * (j % 2) + RES, j // 2, g, :],
                        Ey[:, cb : cb + RES], Ex[:, cb : cb + RES],
                        start=True, stop=True,
                    )

            # ---- per-stroke max (max-norm) ----
            RM = wpool.tile([SP, 32], F32)
            nc.vector.memset(RM[:], 0.0)
            nc.vector.reduce_max(
                RM[:, 0 : NH * BL].rearrange("p (a b) -> p a b", b=BL),
                S_all[:], axis=mybir.AxisListType.X)
            # S -> SBUF copy on Act (idle between the Exps and the Tb tanh;
            # GPSIMD cannot read PSUM on real HW).  Copy is in table set 0.
            S_sb = wpool.tile([SP, NH * BL * RES], F32)
            nc.scalar.activation(
                S_sb[:], S_all[:].rearrange("p a b c -> p (a b c)"), AF.Copy)
            RMT = wpool.tile([SP, 32], F32)
            nc.vector.transpose(RMT[:], RM[:])
            m2 = wpool.tile([NH * BL, 2], F32)
            nc.vector.reduce_max(m2[:, 0:1], RMT[0 : NH * BL, :],
                                 axis=mybir.AxisListType.X)
            nc.vector.reduce_max(m2[:, 1:2], RMT[32 : 32 + NH * BL, :],
                                 axis=mybir.AxisListType.X)
            # r = 1 / (m + eps/zp)  ==  zp / (zp*m + eps); the reciprocal
            # reads a broadcast view and writes the rT matmul operand
            # directly (fuses the broadcast copy away)
            r2 = wpool.tile([NH * BL, 2], F32)
            nc.vector.tensor_tensor(r2[:], m2[:], ezi[:, 0:2], op=ALU.add)
            rT = wpool.tile([NH * BL, 2, 32], BF16)
            nc.vector.memset(rT[:], 0.0)
            with nc.allow_low_precision(reason="rT was bf16 via tensor_copy "
                                        "before; identical numerics"):
                nc.vector.reciprocal(
                    rT[:, :, 0:RES],
                    r2[:, :, None].broadcast_to([NH * BL, 2, RES]))
            R_all = spool.tile([SP, NH * BL * RES], F32, tag="ep")
            nc.tensor.matmul(
                R_all[:], rT[:].rearrange("p a b -> p (a b)"), deltaH[:],
                start=True, stop=True,
            )

            # ---- gate, tanh-norm, stroke-sum, final tanh-norm ----
            gated = wpool.tile([SP, NH * BL * RES], F32)
            nc.vector.tensor_tensor(gated[:], R_all[:], S_sb[:], op=ALU.mult)
            Tb = wpool.tile([SP, NH, BL * RES], BF16)
            nc.scalar.activation(
                Tb[:].rearrange("p a b -> p (a b)"), gated[:],
                AF.Tanh, bias=zbias[0:SP, :], scale=float(slope_strk),
            )
            kp = spool.tile([RES, BL * RES], F32, tag="ep")
            nc.tensor.matmul(kp[:], ksum[:], Tb[:, 0, :], start=True, stop=False)
            nc.tensor.matmul(kp[:], ksum[:], Tb[:, 1, :], start=False, stop=True)
            nc.scalar.activation(
                at[0:RES, 0].rearrange("p b x -> p (b x)"), kp[:],
                AF.Tanh, bias=zbias[0:RES, :], scale=float(slope) * post1,
            )
            # tail is just trigger -> transfer -> completion sem: the 625ns
            # HWDGE + 650ns DGE-DMA delay of the plain DMA path vanish.
            # The placeholder osem wait is repointed post-compile at the Act
            # engine-tick sem (activations only carry ONE hw sem update, so
            # a then_inc on the final act is rejected by codegen).  The SP
            # sem_inc only satisfies the scheduler's deadlock check.
            nc.sync.sem_inc(osem, 1)
            nc.gpsimd.trigger_dma(count=None, queue_num=0)._wait_ge(osem, 1)

    nc.compile()
    # A SWDGE prep's single hardware completion-sem slot must hold the
    # DMASW lane sem Tile pointed the data consumers at; the sem= handle we
    # passed only placates the API.  Repoint update[0] accordingly.
    # (After compile: the sem waits materialize during compile().)
    _patch_prep_sems(nc)
    return nc


def _patch_prep_sems(nc):
    from concourse.tile_sem_assignment import PROC_NAME_TO_IDX

    idx_to_lane = {v: k for k, v in PROC_NAME_TO_IDX.items()
                   if k.startswith("DMASW")}
    fn = nc.m.functions[0]
    lane_ids = {}
    for b in fn.blocks:
        for i in b.instructions:
            si = i.sync_info
            if si is None:
                continue
            for w in list(si.on_wait) + list(si.on_update):
                nm = str(w.ant_name or "")
                if nm.startswith("DMASW"):
                    lane_ids[nm.split("_")[0]] = (w.id, w.ant_name)
    kv_lane_sem = None
    for b in fn.blocks:
        for i in b.instructions:
            if type(i).__name__ in ("InstDMAGatherAnt", "InstKVWritebackAnt"):
                lane = idx_to_lane[i.bass_scheduled_proc]
                sid, sname = lane_ids[lane]
                si = i.sync_info
                u0 = list(si.on_update)[0]
                assert str(u0.ant_name) == "out_dma", u0
                u0.id = sid
                u0.ant_name = sname
                if type(i).__name__ == "InstKVWritebackAnt":
                    kv_lane_sem = str(sname)
    # Tile guards the final-act write of `at` behind the out-DMA completion
    # (WAR vs the early desc-prep's deferred read).  That read really happens
    # at the trigger, strictly after the act, so the guard is a false cycle:
    # act -> trigger -> DMA -> act.  Defuse the Activation-side guard.
    assert kv_lane_sem is not None
    for b in fn.blocks:
        for i in b.instructions:
            if (type(i).__name__ == "InstEventSemaphore"
                    and i.engine == mybir.EngineType.Activation):
                si = i.sync_info
                for w in (si.on_wait if si else []):
                    if str(w.ant_name) == kv_lane_sem:
                        w.wait_value = 0
    # Repoint the out-trigger's placeholder osem wait at the Act engine-tick
    # sem value reached by the FINAL activation (activations carry only one
    # hw sem update, so the act itself cannot inc a user sem).
    act_sem = None   # (id, name) of the Act engine tick sem
    act_ticks = 0
    ticks_at_last_act = 0
    for b in fn.blocks:
        for i in b.instructions:
            si = i.sync_info
            if si is None:
                continue
            for u in si.on_update:
                nm = str(u.ant_name or "")
                if nm.startswith("Activation_"):
                    act_sem = (u.id, u.ant_name)
                    act_ticks += int(u.update_value or 1)
                    if type(i).__name__ == "InstActivation":
                        ticks_at_last_act = act_ticks
    assert act_sem is not None and ticks_at_last_act > 0
    for b in fn.blocks:
        for i in b.instructions:
            if type(i).__name__ == "InstTriggerDma":
                si = i.sync_info
                for w in (si.on_wait if si else []):
                    if str(w.ant_name) == "out_ready":
                        w.id = act_sem[0]
                        w.ant_name = act_sem[1]
                        w.wait_value = ticks_at_last_act


_CACHE = {}


def _get_program(sigma, slope_strk, slope):
    key = (float(sigma), float(slope_strk), float(slope))
    if key not in _CACHE:
        _CACHE[key] = _build_program(*key)
    return _CACHE[key]


def _basis(steps):
    t = np.linspace(0.0, 1.0, steps, dtype=np.float64)[:, None]
    i = np.arange(PTS, dtype=np.float64)[None, :]
    binom = np.array([comb(PTS - 1, j) for j in range(PTS)],
                     dtype=np.float64)[None, :]
    return binom * (t ** i) * ((1.0 - t) ** (PTS - 1 - i))      # [steps, 5]


def _host_consts():
    grid = np.linspace(0.0, 1.0, RES, dtype=np.float64)
    # gmat rows: [G0; G0; G1; 1; 1; g2h; g2l] (fp16)
    G0 = (-2.0 * grid).astype(np.float16)
    G1 = (-2.0 * grid - G0.astype(np.float64)).astype(np.float16)
    g2 = grid ** 2
    g2h = g2.astype(np.float16)
    g2l = (g2 - g2h.astype(np.float64)).astype(np.float16)
    ones = np.ones(RES, np.float16)
    gmat = np.stack([G0, G0, G1, ones, ones, g2h, g2l])          # [7, 28]
    # deltaH[(h'*8+g'), (h, g, x)] = delta(h,h')*delta(g,g')
    deltaH = np.kron(np.eye(NH * BL, dtype=np.float32),
                     np.ones((1, RES), np.float32))              # [16, 448]
    # ksum64[(j2*32+y), y'] = delta(y,y') for y<28, zero pad rows
    ksum = np.zeros((SP, RES), dtype=np.float32)
    for j2 in range(2):
        ksum[j2 * 32 : j2 * 32 + RES] = np.eye(RES, dtype=np.float32)
    return gmat, deltaH, ksum


def _host_inputs(z_pres, z_what, z_where, sigma):
    """Per-core input dicts: tiny curve linear algebra done host-side."""
    import ml_dtypes

    gmat, deltaH, ksum = _host_consts()
    Bm = _basis(T)                                            # [T, 5]
    inv = 1.0 / (2.0 * sigma * sigma)
    A = (STEPS_REF - 1) / (T - 1)
    w = np.full(T, A, dtype=np.float64)
    w[0] = w[-1] = (1.0 + A) / 2.0
    # fold the per-sample weight into the c^2 rows: per coord the Exp arg
    # gains +0.5*ln(w_t), i.e. c2' = c^2 - 0.5*ln(w_t)/inv
    c2_off = 0.5 * np.log(w) / inv                            # [T]

    s = z_where[..., 0].astype(np.float64)
    shift = z_where[..., 1:3].astype(np.float64)
    pts = z_what.astype(np.float64) * s[..., None, None] + shift[..., None, :]
    curve = np.einsum("tp,bkpd->bktd", Bm, pts)               # [64,4,T,2]

    ksum_bf = ksum.astype(ml_dtypes.bfloat16)
    in_maps = []
    for cidx in range(NCORES):
        sl = slice(cidx * BL, (cidx + 1) * BL)
        cv = curve[sl].reshape(NPAIR, T, 2)                   # [32,T,2]
        lhsy = np.zeros((RPP, LYC), dtype=np.float16)
        lhsx = np.zeros((RPP, LXC), dtype=np.float16)
        for p in range(NPAIR):
            for coord in range(2):
                cc = cv[p, :, coord]                          # [T] f64
                c0 = cc.astype(np.float16)
                c1 = (cc - c0.astype(np.float64)).astype(np.float16)
                c2 = cc ** 2 - c2_off
                c2h = c2.astype(np.float16)
                c2l = (c2 - c2h.astype(np.float64)).astype(np.float16)
                blk = np.stack([c0, c1, c0, c2h, c2l,
                                np.ones(T, np.float16), np.ones(T, np.float16)])
                dst = lhsy if coord == 1 else lhsx
                dst[:, p * T : (p + 1) * T] = blk
        lhsy[0:RPP, C_G : C_G + RES] = gmat
        # deltah payload: [16, 448] selector + ezi [16,2] f32 in 4 bf16 cols
        zp = z_pres[sl].astype(np.float64)                    # [8,4]
        ezi = np.zeros((NH * BL, 2), dtype=np.float32)
        for h in range(NH):
            for g in range(BL):
                for j2 in range(2):
                    ezi[h * BL + g, j2] = EPS / max(zp[g, 2 * h + j2], 1e-30)
        dh = np.zeros((NH * BL, DHC), dtype=ml_dtypes.bfloat16)
        dh[:, 0 : NH * BL * RES] = deltaH.astype(ml_dtypes.bfloat16)
        dh[:, NH * BL * RES : DHC] = ezi.view(np.uint16).view(ml_dtypes.bfloat16)
        m = {"ksum": ksum_bf, "deltah": dh}
        m["lhsy"] = lhsy
        m["lhsx"] = lhsx
        in_maps.append(m)
    return in_maps


def kernel(z_pres, z_what, z_where, sigma, slope_strk, slope):
    z_pres = np.asarray(z_pres, np.float32)
    z_what = np.asarray(z_what, np.float32)
    z_where = np.asarray(z_where, np.float32)
    nc = _get_program(float(sigma), float(slope_strk), float(slope))
    in_maps = _host_inputs(z_pres, z_what, z_where, float(sigma))
    res = run_bass_kernel_spmd(nc, in_maps, core_ids=list(range(NCORES)))
    out = np.concatenate([np.asarray(r["out"], np.float32)[:, 0:RES, 0, :]
                          for r in res.results], axis=0)            # [64,28,28]
    out = out * np.float32(1.0 / tanh(float(slope)))   # post-scale on host
    return out[:, None].astype(np.float32)


# revision 3
# speedup vs baseline: 1.0155x; 1.0155x over previous
"""Bezier-stroke rasterizer (AIR/Guide-style) as a Trainium2 Bass/Tile kernel.

v2 redesign vs the selector-matrix baseline:
- d2 = (c-g)^2 is computed DIRECTLY by the PE as 64 tiny per-pair matmuls
  [7,T]x[7,28]: contraction rows are free (matmul cost ~ output cols), the
  rhs is ONE shared [7,28] constant (G hi/lo, ones, g^2 hi/lo) and the lhs
  is per-pair fp16 hi/lo c-data -> the 118KB selector matrix is gone; the
  critical input DMA drops to ~140KB.
- The gaussian is Exp(-inv*d2 + 0.5*ln w_t): Exp lives in act table set 0
  together with Tanh/Copy, so the WHOLE kernel uses one table set -> the
  1283ns mid-kernel table reload and its dummy-tanh hoist are gone.  The
  trapezoid weight folds into Exp's per-partition bias (0.5*ln w on both
  coords) -> the DVE weight-multiply is gone too.
- T=128 curve samples (quadrature error ~1e-8 for worst-case stroke
  lengths); trapezoid weights fold into the host-side c^2 rows.
- Output leaves via a Pool SWDGE kv_writeback whose descriptors are
  prepped during the input-DMA wait and merely TRIGGERED after the final
  activation: the 625ns HWDGE + 650ns DGE delay vanish from the tail.

Sharding: pure data parallel, 8 batches per core across 8 NeuronCores.
"""

import sys
import numpy as np
from math import comb, tanh

sys.path.insert(0, "/opt/trn_rl_repo")

from concourse import bass, bacc, tile, mybir  # noqa: E402
from concourse.bass_utils import run_bass_kernel_spmd  # noqa: E402

BS, K, PTS, RES = 64, 4, 5, 28
STEPS_REF = 500
T = 128                     # curve samples on device
NCORES = 8
BL = BS // NCORES           # local batches per core = 8
NPAIR = BL * K              # (batch, stroke) pairs per core = 32
NA = 16                     # pairs in PSUM bank0 chunk (= batches g 0..3)
NB = NPAIR - NA             # pairs in bank1 chunk (= batches g 4..7)
RPP = 7                     # lhs rows per pair
NH = 2                      # stroke column-halves (j//2)
SP = 64                     # stroke partition extent (2 x 32-padded)
EPS = 1e-6
F32 = mybir.dt.float32
F16 = mybir.dt.float16
BF16 = mybir.dt.bfloat16
AF = mybir.ActivationFunctionType
ALU = mybir.AluOpType

# lhs layout: two tensors on partitions 0-6 (matmul base-partition rule):
#   lhsy [7, LYC]: pair p's y c-rows at cols p*T; gmat at col 32*T
#   lhsx [7, 32*T]: pair p's x c-rows at cols p*T
# The trapezoid weight folds into the host-side c^2 rows (c2' = c^2 -
# 0.5*ln(w_t)/inv), so the Exp bias is plain zero.  EPS/zp rides in 4
# extra bf16 columns of deltah (bitcast to f32).
C_G = NPAIR * T
LYC = C_G + RES
LXC = NPAIR * T
DHC = NH * BL * RES + 4     # deltah cols + 4 bf16 cols holding ezi f32


def _pair_col(p):
    """column base of pair p in the [*, 1024] d2/E tiles"""
    return 28 * p if p < NA else 512 + 28 * (p - NA)


def _build_program(sigma, slope_strk, slope):
    inv = 1.0 / (2.0 * sigma * sigma)
    post1 = 1.0 / tanh(slope_strk)

    nc = bacc.Bacc(None, target_bir_lowering=False)

    lhsy_d = nc.dram_tensor("lhsy", [RPP, LYC], F16, kind="ExternalInput")
    lhsx_d = nc.dram_tensor("lhsx", [RPP, LXC], F16, kind="ExternalInput")
    dl_d = nc.dram_tensor("deltah", [NH * BL, DHC], BF16,
                          kind="ExternalInput")
    ks_d = nc.dram_tensor("ksum", [SP, RES], BF16, kind="ExternalInput")
    out_d = nc.dram_tensor("out", [BL, 128, 1, RES], BF16,
                           kind="ExternalOutput")

    with tile.TileContext(nc) as tc:
        with (
            tc.tile_pool(name="const", bufs=1) as cpool,
            tc.tile_pool(name="work", bufs=1) as wpool,
            tc.tile_pool(name="dsq", bufs=2, space="PSUM") as dpool,
            tc.tile_pool(name="sp", bufs=1, space="PSUM") as spool,
        ):
            # ---- input DMAs (SP queue: lhsy critical-first) ----
            lhsy = cpool.tile([RPP, LYC], F16)
            nc.sync.dma_start(lhsy[:], lhsy_d[:])
            lhsx = cpool.tile([RPP, LXC], F16)
            nc.sync.dma_start(lhsx[:], lhsx_d[:])
            deltaHz = cpool.tile([NH * BL, DHC], BF16)
            nc.sync.dma_start(deltaHz[:], dl_d[:])
            ksum = cpool.tile([SP, RES], BF16)
            nc.sync.dma_start(ksum[:], ks_d[:])

            gmat = lhsy[0:RPP, C_G : C_G + RES]
            deltaH = deltaHz[:, 0 : NH * BL * RES]
            ezi = deltaHz[:, NH * BL * RES : DHC].bitcast(F32)  # [16, 2] f32

            # ---- act-table hoist ----
            scratch = wpool.tile([128, 512], BF16)
            nc.vector.memset(scratch[:], 0.0)
            zbias = wpool.tile([128, 1], F32)
            nc.vector.memset(zbias[:], 0.0)
            expsc = wpool.tile([1, 8], F32)
            # dummy Exp: places the single set-0 table load in the DMA wait
            nc.scalar.activation(expsc[:], scratch[0:1, 0:8], AF.Exp,
                                 bias=zbias[0:1, :])

            # output kv_writeback: desc-gen on the idle Pool engine NOW --
            # `at` has no producer yet so the prep carries no data wait; the
            # trigger at the end is repointed (post-compile) at the final
            # activation's engine tick.
            at = wpool.tile([128, 1, BL, RES], BF16)
            kidx = wpool.tile([128, BL], mybir.dt.int32)
            nc.gpsimd.memset(kidx[:], 0)
            ksem = nc.alloc_semaphore("out_dma")
            osem = nc.alloc_semaphore("out_ready")
            nc.gpsimd.kv_writeback(out_d[:], at[:], kidx[:],
                                   prepare_only=True, sem=ksem, queue_num=0)

            # ---- d2 matmuls: 64 x [7,T]x[7,28], then Exp per coord ----
            # d2 tile [T, 1024]: pairs 0:16 (batches g<4) in bank0 cols
            # 0:448, pairs 16:32 in bank1 cols 512:960.  The acts read a
            # strided [T, 2, 448] view so the pad columns cost nothing.
            # ln(w) is folded into the host-side c^2 rows -> bias is zero.
            def _emit_mms(lt, d2, lo, hi):
                for p in range(lo, hi):
                    cb = _pair_col(p)
                    nc.tensor.matmul(
                        d2[:, cb : cb + RES],
                        lt[0:RPP, p * T : (p + 1) * T],
                        gmat,
                        start=True, stop=True,
                    )

            def _v(tile_):
                return tile_[:].rearrange(
                    "p (a b) -> p a b", a=2, b=512)[:, :, 0:448]

            d2y = dpool.tile([T, 1024], F32, tag="d2")
            _emit_mms(lhsy, d2y, 0, NPAIR)
            Ey = wpool.tile([T, 1024], F16, name="Ey")
            nc.scalar.activation(_v(Ey), _v(d2y),
                                 AF.Exp, bias=zbias[0:T, :], scale=-float(inv))
            d2x = dpool.tile([T, 1024], F32, tag="d2")
            Ex = wpool.tile([T, 1024], F16, name="Ex")
            _emit_mms(lhsx, d2x, 0, NPAIR)
            nc.scalar.activation(_v(Ex), _v(d2x),
                                 AF.Exp, bias=zbias[0:T, :], scale=-float(inv))

            # ---- stroke maps: 32 garbage-free [28,28] matmuls ----
            # stroke (g, j) = pair p = 4g+j -> partition base 32*(j%2),
            # column (j//2, g, :).  Chunk g<4 first: its strokes only need
            # the first x-act, and its row-max runs while chunk B strokes
            # are still in flight.
            S_all = spool.tile([SP, NH, BL, RES], F32, tag="S")
            nc.vector.memset(S_all[:], 0.0)
            for g in range(BL):
                for j in range(K):
                    p = 4 * g + j
                    cb = _pair_col(p)
                    nc.tensor.matmul(
                        S_all[32 * (j % 2) : 32 * (j % 2) + RES, j // 2, g, :],
                        Ey[:, cb : cb + RES], Ex[:, cb : cb + RES],
                        start=True, stop=True,
                    )

            # ---- per-stroke max (max-norm) ----
            RM = wpool.tile([SP, 32], F32)
            nc.vector.memset(RM[:], 0.0)
            nc.vector.reduce_max(
                RM[:, 0 : NH * BL].rearrange("p (a b) -> p a b", b=BL),
                S_all[:], axis=mybir.AxisListType.X)
            # S -> SBUF copy on Act (idle between the Exps and the Tb tanh;
            # GPSIMD cannot read PSUM on real HW).  Copy is in table set 0.
            S_sb = wpool.tile([SP, NH * BL * RES], F32)
            nc.scalar.activation(
                S_sb[:], S_all[:].rearrange("p a b c -> p (a b c)"), AF.Copy)
            RMT = wpool.tile([SP, 32], F32)
            nc.vector.transpose(RMT[:], RM[:])
            m2 = wpool.tile([NH * BL, 2], F32)
            nc.vector.reduce_max(m2[:, 0:1], RMT[0 : NH * BL, :],
                                 axis=mybir.AxisListType.X)
            nc.vector.reduce_max(m2[:, 1:2], RMT[32 : 32 + NH * BL, :],
                                 axis=mybir.AxisListType.X)
            # r = 1 / (m + eps/zp)  ==  zp / (zp*m + eps); the reciprocal
            # reads a broadcast view and writes the rT matmul operand
            # directly (fuses the broadcast copy away)
            r2 = wpool.tile([NH * BL, 2], F32)
            nc.vector.tensor_tensor(r2[:], m2[:], ezi[:, 0:2], op=ALU.add)
            rT = wpool.tile([NH * BL, 2, 32], BF16)
            nc.vector.memset(rT[:], 0.0)
            with nc.allow_low_precision(reason="rT was bf16 via tensor_copy "
                                        "before; identical numerics"):
                nc.vector.reciprocal(
                    rT[:, :, 0:RES],
                    r2[:, :, None].broadcast_to([NH * BL, 2, RES]))
            R_all = spool.tile([SP, NH * BL * RES], F32, tag="ep")
            nc.tensor.matmul(
                R_all[:], rT[:].rearrange("p a b -> p (a b)"), deltaH[:],
                start=True, stop=True,
            )

            # ---- gate, tanh-norm, stroke-sum, final tanh-norm ----
            gated = wpool.tile([SP, NH * BL * RES], F32)
            nc.vector.tensor_tensor(gated[:], R_all[:], S_sb[:], op=ALU.mult)
            Tb = wpool.tile([SP, NH, BL * RES], BF16)
            nc.scalar.activation(
                Tb[:].rearrange("p a b -> p (a b)"), gated[:],
                AF.Tanh, bias=zbias[0:SP, :], scale=float(slope_strk),
            )
            kp = spool.tile([RES, BL * RES], F32, tag="ep")
            nc.tensor.matmul(kp[:], ksum[:], Tb[:, 0, :], start=True, stop=False)
            nc.tensor.matmul(kp[:], ksum[:], Tb[:, 1, :], start=False, stop=True)
            nc.scalar.activation(
                at[0:RES, 0].rearrange("p b x -> p (b x)"), kp[:],
                AF.Tanh, bias=zbias[0:RES, :], scale=float(slope) * post1,
            )
            # tail is just trigger -> transfer -> completion sem: the 625ns
            # HWDGE + 650ns DGE-DMA delay of the plain DMA path vanish.
            # The placeholder osem wait is repointed post-compile at the Act
            # engine-tick sem (activations only carry ONE hw sem update, so
            # a then_inc on the final act is rejected by codegen).  The SP
            # sem_inc only satisfies the scheduler's deadlock check.
            nc.sync.sem_inc(osem, 1)
            nc.gpsimd.trigger_dma(count=None, queue_num=0)._wait_ge(osem, 1)

    nc.compile()
    # A SWDGE prep's single hardware completion-sem slot must hold the
    # DMASW lane sem Tile pointed the data consumers at; the sem= handle we
    # passed only placates the API.  Repoint update[0] accordingly.
    # (After compile: the sem waits materialize during compile().)
    _patch_prep_sems(nc)
    return nc


def _patch_prep_sems(nc):
    from concourse.tile_sem_assignment import PROC_NAME_TO_IDX

    idx_to_lane = {v: k for k, v in PROC_NAME_TO_IDX.items()
                   if k.startswith("DMASW")}
    fn = nc.m.functions[0]
    lane_ids = {}
    for b in fn.blocks:
        for i in b.instructions:
            si = i.sync_info
            if si is None:
                continue
            for w in list(si.on_wait) + list(si.on_update):
                nm = str(w.ant_name or "")
                if nm.startswith("DMASW"):
                    lane_ids[nm.split("_")[0]] = (w.id, w.ant_name)
    kv_lane_sem = None
    for b in fn.blocks:
        for i in b.instructions:
            if type(i).__name__ in ("InstDMAGatherAnt", "InstKVWritebackAnt"):
                lane = idx_to_lane[i.bass_scheduled_proc]
                sid, sname = lane_ids[lane]
                si = i.sync_info
                u0 = list(si.on_update)[0]
                assert str(u0.ant_name) == "out_dma", u0
                u0.id = sid
                u0.ant_name = sname
                if type(i).__name__ == "InstKVWritebackAnt":
                    kv_lane_sem = str(sname)
    # Tile guards the final-act write of `at` behind the out-DMA completion
    # (WAR vs the early desc-prep's deferred read).  That read really happens
    # at the trigger, strictly after the act, so the guard is a false cycle:
    # act -> trigger -> DMA -> act.  Defuse the Activation-side guard.
    assert kv_lane_sem is not None
    for b in fn.blocks:
        for i in b.instructions:
            if (type(i).__name__ == "InstEventSemaphore"
                    and i.engine == mybir.EngineType.Activation):
                si = i.sync_info
                for w in (si.on_wait if si else []):
                    if str(w.ant_name) == kv_lane_sem:
                        w.wait_value = 0
    # Repoint the out-trigger's placeholder osem wait at the Act engine-tick
    # sem value reached by the FINAL activation (activations carry only one
    # hw sem update, so the act itself cannot inc a user sem).
    act_sem = None   # (id, name) of the Act engine tick sem
    act_ticks = 0
    ticks_at_last_act = 0
    for b in fn.blocks:
        for i in b.instructions:
            si = i.sync_info
            if si is None:
                continue
            for u in si.on_update:
                nm = str(u.ant_name or "")
                if nm.startswith("Activation_"):
                    act_sem = (u.id, u.ant_name)
                    act_ticks += int(u.update_value or 1)
                    if type(i).__name__ == "InstActivation":
                        ticks_at_last_act = act_ticks
    assert act_sem is not None and ticks_at_last_act > 0
    for b in fn.blocks:
        for i in b.instructions:
            if type(i).__name__ == "InstTriggerDma":
                si = i.sync_info
                for w in (si.on_wait if si else []):
                    if str(w.ant_name) == "out_ready":
                        w.id = act_sem[0]
                        w.ant_name = act_sem[1]
                        w.wait_value = ticks_at_last_act


_CACHE = {}


def _get_program(sigma, slope_strk, slope):
    key = (float(sigma), float(slope_strk), float(slope))
    if key not in _CACHE:
        _CACHE[key] = _build_program(*key)
    return _CACHE[key]


def _basis(steps):
    t = np.linspace(0.0, 1.0, steps, dtype=np.float64)[:, None]
    i = np.arange(PTS, dtype=np.float64)[None, :]
    binom = np.array([comb(PTS - 1, j) for j in range(PTS)],
                     dtype=np.float64)[None, :]
    return binom * (t ** i) * ((1.0 - t) ** (PTS - 1 - i))      # [steps, 5]


def _host_consts():
    grid = np.linspace(0.0, 1.0, RES, dtype=np.float64)
    # gmat rows: [G0; G0; G1; 1; 1; g2h; g2l] (fp16)
    G0 = (-2.0 * grid).astype(np.float16)
    G1 = (-2.0 * grid - G0.astype(np.float64)).astype(np.float16)
    g2 = grid ** 2
    g2h = g2.astype(np.float16)
    g2l = (g2 - g2h.astype(np.float64)).astype(np.float16)
    ones = np.ones(RES, np.float16)
    gmat = np.stack([G0, G0, G1, ones, ones, g2h, g2l])          # [7, 28]
    # deltaH[(h'*8+g'), (h, g, x)] = delta(h,h')*delta(g,g')
    deltaH = np.kron(np.eye(NH * BL, dtype=np.float32),
                     np.ones((1, RES), np.float32))              # [16, 448]
    # ksum64[(j2*32+y), y'] = delta(y,y') for y<28, zero pad rows
    ksum = np.zeros((SP, RES), dtype=np.float32)
    for j2 in range(2):
        ksum[j2 * 32 : j2 * 32 + RES] = np.eye(RES, dtype=np.float32)
    return gmat, deltaH, ksum


def _host_inputs(z_pres, z_what, z_where, sigma):
    """Per-core input dicts: tiny curve linear algebra done host-side."""
    import ml_dtypes

    gmat, deltaH, ksum = _host_consts()
    Bm = _basis(T)                                            # [T, 5]
    inv = 1.0 / (2.0 * sigma * sigma)
    A = (STEPS_REF - 1) / (T - 1)
    w = np.full(T, A, dtype=np.float64)
    w[0] = w[-1] = (1.0 + A) / 2.0
    # fold the per-sample weight into the c^2 rows: per coord the Exp arg
    # gains +0.5*ln(w_t), i.e. c2' = c^2 - 0.5*ln(w_t)/inv
    c2_off = 0.5 * np.log(w) / inv                            # [T]

    s = z_where[..., 0].astype(np.float64)
    shift = z_where[..., 1:3].astype(np.float64)
    pts = z_what.astype(np.float64) * s[..., None, None] + shift[..., None, :]
    curve = np.einsum("tp,bkpd->bktd", Bm, pts)               # [64,4,T,2]

    ksum_bf = ksum.astype(ml_dtypes.bfloat16)
    in_maps = []
    for cidx in range(NCORES):
        sl = slice(cidx * BL, (cidx + 1) * BL)
        cv = curve[sl].reshape(NPAIR, T, 2)                   # [32,T,2]
        lhsy = np.zeros((RPP, LYC), dtype=np.float16)
        lhsx = np.zeros((RPP, LXC), dtype=np.float16)
        for p in range(NPAIR):
            for coord in range(2):
                cc = cv[p, :, coord]                          # [T] f64
                c0 = cc.astype(np.float16)
                c1 = (cc - c0.astype(np.float64)).astype(np.float16)
                c2 = cc ** 2 - c2_off
                c2h = c2.astype(np.float16)
                c2l = (c2 - c2h.astype(np.float64)).astype(np.float16)
                blk = np.stack([c0, c1, c0, c2h, c2l,
                                np.ones(T, np.float16), np.ones(T, np.float16)])
                dst = lhsy if coord == 1 else lhsx
                dst[:, p * T : (p + 1) * T] = blk
        lhsy[0:RPP, C_G : C_G + RES] = gmat
        # deltah payload: [16, 448] selector + ezi [16,2] f32 in 4 bf16 cols
        zp = z_pres[sl].astype(np.float64)                    # [8,4]
        ezi = np.zeros((NH * BL, 2), dtype=np.float32)
        for h in range(NH):
            for g in range(BL):
                for j2 in range(2):
                    ezi[h * BL + g, j2] = EPS / max(zp[g, 2 * h + j2], 1e-30)
        dh = np.zeros((NH * BL, DHC), dtype=ml_dtypes.bfloat16)
        dh[:, 0 : NH * BL * RES] = deltaH.astype(ml_dtypes.bfloat16)
        dh[:, NH * BL * RES : DHC] = ezi.view(np.uint16).view(ml_dtypes.bfloat16)
        m = {"ksum": ksum_bf, "deltah": dh}
        m["lhsy"] = lhsy
        m["lhsx"] = lhsx
        in_maps.append(m)
    return in_maps


def kernel(z_pres, z_what, z_where, sigma, slope_strk, slope):
    z_pres = np.asarray(z_pres, np.float32)
    z_what = np.asarray(z_what, np.float32)
    z_where = np.asarray(z_where, np.float32)
    nc = _get_program(float(sigma), float(slope_strk), float(slope))
    in_maps = _host_inputs(z_pres, z_what, z_where, float(sigma))
    res = run_bass_kernel_spmd(nc, in_maps, core_ids=list(range(NCORES)))
    out = np.concatenate([np.asarray(r["out"], np.float32)[:, 0:RES, 0, :]
                          for r in res.results], axis=0)            # [64,28,28]
    out = out * np.float32(1.0 / tanh(float(slope)))   # post-scale on host
    return out[:, None].astype(np.float32)


# revision 4
# speedup vs baseline: 1.0282x; 1.0125x over previous
"""Bezier-stroke rasterizer (AIR/Guide-style) as a Trainium2 Bass/Tile kernel.

v2 redesign vs the selector-matrix baseline:
- d2 = (c-g)^2 is computed DIRECTLY by the PE as 64 tiny per-pair matmuls
  [7,T]x[7,28]: contraction rows are free (matmul cost ~ output cols), the
  rhs is ONE shared [7,28] constant (G hi/lo, ones, g^2 hi/lo) and the lhs
  is per-pair fp16 hi/lo c-data -> the 118KB selector matrix is gone; the
  critical input DMA drops to ~140KB.
- The gaussian is Exp(-inv*d2 + 0.5*ln w_t): Exp lives in act table set 0
  together with Tanh/Copy, so the WHOLE kernel uses one table set -> the
  1283ns mid-kernel table reload and its dummy-tanh hoist are gone.  The
  trapezoid weight folds into Exp's per-partition bias (0.5*ln w on both
  coords) -> the DVE weight-multiply is gone too.
- T=128 curve samples (quadrature error ~1e-8 for worst-case stroke
  lengths); trapezoid weights fold into the host-side c^2 rows.
- Output leaves via a Pool SWDGE kv_writeback whose descriptors are
  prepped during the input-DMA wait and merely TRIGGERED after the final
  activation: the 625ns HWDGE + 650ns DGE delay vanish from the tail.

Sharding: pure data parallel, 8 batches per core across 8 NeuronCores.
"""

import sys
import numpy as np
from math import comb, tanh

sys.path.insert(0, "/opt/trn_rl_repo")

from concourse import bass, bacc, tile, mybir  # noqa: E402
from concourse.bass_utils import run_bass_kernel_spmd  # noqa: E402

BS, K, PTS, RES = 64, 4, 5, 28
STEPS_REF = 500
T = 128                     # curve samples on device
NCORES = 8
BL = BS // NCORES           # local batches per core = 8
NPAIR = BL * K              # (batch, stroke) pairs per core = 32
NA = 16                     # pairs in PSUM bank0 chunk (= batches g 0..3)
NB = NPAIR - NA             # pairs in bank1 chunk (= batches g 4..7)
RPP = 7                     # lhs rows per pair
NH = 2                      # stroke column-halves (j//2)
SP = 64                     # stroke partition extent (2 x 32-padded)
EPS = 1e-6
F32 = mybir.dt.float32
F16 = mybir.dt.float16
BF16 = mybir.dt.bfloat16
AF = mybir.ActivationFunctionType
ALU = mybir.AluOpType

# lhs layout: two tensors on partitions 0-6 (matmul base-partition rule):
#   lhsy [7, LYC]: pair p's y c-rows at cols p*T; gmat at col 32*T
#   lhsx [7, 32*T]: pair p's x c-rows at cols p*T
# The trapezoid weight folds into the host-side c^2 rows (c2' = c^2 -
# 0.5*ln(w_t)/inv), so the Exp bias is plain zero.  EPS/zp rides in 4
# extra bf16 columns of deltah (bitcast to f32).
C_G = NPAIR * T
LYC = C_G + RES
LXC = NPAIR * T
DHC = NH * BL * RES + 4     # deltah cols + 4 bf16 cols holding ezi f32


def _pair_col(p):
    """column base of pair p in the [*, 1024] d2/E tiles"""
    return 28 * p if p < NA else 512 + 28 * (p - NA)


def _build_program(sigma, slope_strk, slope):
    inv = 1.0 / (2.0 * sigma * sigma)
    post1 = 1.0 / tanh(slope_strk)

    nc = bacc.Bacc(None, target_bir_lowering=False)

    lhsy_d = nc.dram_tensor("lhsy", [RPP, LYC], F16, kind="ExternalInput")
    lhsx_d = nc.dram_tensor("lhsx", [RPP, LXC], F16, kind="ExternalInput")
    dl_d = nc.dram_tensor("deltah", [NH * BL, DHC], BF16,
                          kind="ExternalInput")
    ks_d = nc.dram_tensor("ksum", [SP, RES], BF16, kind="ExternalInput")
    out_d = nc.dram_tensor("out", [BL, 128, 1, RES], BF16,
                           kind="ExternalOutput")

    with tile.TileContext(nc) as tc:
        with (
            tc.tile_pool(name="const", bufs=1) as cpool,
            tc.tile_pool(name="work", bufs=1) as wpool,
            tc.tile_pool(name="dsq", bufs=2, space="PSUM") as dpool,
            tc.tile_pool(name="sp", bufs=1, space="PSUM") as spool,
        ):
            # ---- input DMAs (SP queue: lhsy critical-first) ----
            lhsy = cpool.tile([RPP, LYC], F16)
            nc.sync.dma_start(lhsy[:], lhsy_d[:])
            lhsx = cpool.tile([RPP, LXC], F16)
            nc.sync.dma_start(lhsx[:], lhsx_d[:])
            deltaHz = cpool.tile([NH * BL, DHC], BF16)
            nc.sync.dma_start(deltaHz[:], dl_d[:])
            ksum = cpool.tile([SP, RES], BF16)
            nc.sync.dma_start(ksum[:], ks_d[:])

            gmat = lhsy[0:RPP, C_G : C_G + RES]
            deltaH = deltaHz[:, 0 : NH * BL * RES]
            ezi = deltaHz[:, NH * BL * RES : DHC].bitcast(F32)  # [16, 2] f32

            # ---- act-table hoist ----
            scratch = wpool.tile([128, 512], BF16)
            nc.vector.memset(scratch[:], 0.0)
            zbias = wpool.tile([128, 1], F32)
            nc.vector.memset(zbias[:], 0.0)
            expsc = wpool.tile([1, 8], F32)
            # dummy Exp: places the single set-0 table load in the DMA wait
            nc.scalar.activation(expsc[:], scratch[0:1, 0:8], AF.Exp,
                                 bias=zbias[0:1, :])

            # output kv_writeback: desc-gen on the idle Pool engine NOW --
            # `at` has no producer yet so the prep carries no data wait; the
            # trigger at the end is repointed (post-compile) at the final
            # activation's engine tick.
            at = wpool.tile([128, 1, BL, RES], BF16)
            kidx = wpool.tile([128, BL], mybir.dt.int32)
            nc.gpsimd.memset(kidx[:], 0)
            ksem = nc.alloc_semaphore("out_dma")
            osem = nc.alloc_semaphore("out_ready")
            nc.gpsimd.kv_writeback(out_d[:], at[:], kidx[:],
                                   prepare_only=True, sem=ksem, queue_num=0)

            # ---- d2 matmuls: 64 x [7,T]x[7,28], then Exp per coord ----
            # d2 tile [T, 1024]: pairs 0:16 (batches g<4) in bank0 cols
            # 0:448, pairs 16:32 in bank1 cols 512:960.  The acts read a
            # strided [T, 2, 448] view so the pad columns cost nothing.
            # ln(w) is folded into the host-side c^2 rows -> bias is zero.
            def _emit_mms(lt, d2, lo, hi):
                for p in range(lo, hi):
                    cb = _pair_col(p)
                    nc.tensor.matmul(
                        d2[:, cb : cb + RES],
                        lt[0:RPP, p * T : (p + 1) * T],
                        gmat,
                        start=True, stop=True,
                    )

            def _v(tile_):
                return tile_[:].rearrange(
                    "p (a b) -> p a b", a=2, b=512)[:, :, 0:448]

            d2y = dpool.tile([T, 1024], F32, tag="d2")
            _emit_mms(lhsy, d2y, 0, NPAIR)
            Ey = wpool.tile([T, 1024], F16, name="Ey")
            nc.scalar.activation(_v(Ey), _v(d2y),
                                 AF.Exp, bias=zbias[0:T, :], scale=-float(inv))
            d2x = dpool.tile([T, 1024], F32, tag="d2")
            Ex = wpool.tile([T, 1024], F16, name="Ex")
            _emit_mms(lhsx, d2x, 0, NPAIR)
            nc.scalar.activation(_v(Ex), _v(d2x),
                                 AF.Exp, bias=zbias[0:T, :], scale=-float(inv))

            # ---- stroke maps: 32 garbage-free [28,28] matmuls ----
            # stroke (g, j) = pair p = 4g+j -> partition base 32*(j%2),
            # column (j//2, g, :).  Chunk g<4 first: its strokes only need
            # the first x-act, and its row-max runs while chunk B strokes
            # are still in flight.
            S_all = spool.tile([SP, NH, BL, RES], F32, tag="S")
            nc.vector.memset(S_all[:], 0.0)
            for g in range(BL):
                for j in range(K):
                    p = 4 * g + j
                    cb = _pair_col(p)
                    nc.tensor.matmul(
                        S_all[32 * (j % 2) : 32 * (j % 2) + RES, j // 2, g, :],
                        Ey[:, cb : cb + RES], Ex[:, cb : cb + RES],
                        start=True, stop=True,
                    )

            # ---- per-stroke max (max-norm) ----
            RM = wpool.tile([SP, 32], F32)
            nc.vector.memset(RM[:], 0.0)
            nc.vector.reduce_max(
                RM[:, 0 : NH * BL].rearrange("p (a b) -> p a b", b=BL),
                S_all[:], axis=mybir.AxisListType.X)
            # S -> SBUF copy on Act (idle between the Exps and the Tb tanh;
            # GPSIMD cannot read PSUM on real HW).  Copy is in table set 0.
            S_sb = wpool.tile([SP, NH * BL * RES], F32)
            nc.scalar.activation(
                S_sb[:], S_all[:].rearrange("p a b c -> p (a b c)"), AF.Copy)
            # transpose fused into the partition-dim reduces: one DVE op per
            # 32-block instead of transpose + 2 reduces
            m2f = wpool.tile([32, 2], F32)
            nc.vector.reduce_max(m2f[:, 0:1], RM[0:32, :],
                                 axis=mybir.AxisListType.X,
                                 apply_transpose=True)
            nc.vector.reduce_max(m2f[:, 1:2], RM[32:64, :],
                                 axis=mybir.AxisListType.X,
                                 apply_transpose=True)
            m2 = m2f[0 : NH * BL, :]
            # r = 1 / (m + eps/zp)  ==  zp / (zp*m + eps); the reciprocal
            # reads a broadcast view and writes the rT matmul operand
            # directly (fuses the broadcast copy away)
            r2 = wpool.tile([NH * BL, 2], F32)
            nc.vector.tensor_tensor(r2[:], m2, ezi[:, 0:2], op=ALU.add)
            rT = wpool.tile([NH * BL, 2, 32], BF16)
            nc.vector.memset(rT[:], 0.0)
            with nc.allow_low_precision(reason="rT was bf16 via tensor_copy "
                                        "before; identical numerics"):
                nc.vector.reciprocal(
                    rT[:, :, 0:RES],
                    r2[:, :, None].broadcast_to([NH * BL, 2, RES]))
            R_all = spool.tile([SP, NH * BL * RES], F32, tag="ep")
            nc.tensor.matmul(
                R_all[:], rT[:].rearrange("p a b -> p (a b)"), deltaH[:],
                start=True, stop=True,
            )

            # ---- gate, tanh-norm, stroke-sum, final tanh-norm ----
            gated = wpool.tile([SP, NH * BL * RES], F32)
            nc.vector.tensor_tensor(gated[:], R_all[:], S_sb[:], op=ALU.mult)
            Tb = wpool.tile([SP, NH, BL * RES], BF16)
            nc.scalar.activation(
                Tb[:].rearrange("p a b -> p (a b)"), gated[:],
                AF.Tanh, bias=zbias[0:SP, :], scale=float(slope_strk),
            )
            kp = spool.tile([RES, BL * RES], F32, tag="ep")
            nc.tensor.matmul(kp[:], ksum[:], Tb[:, 0, :], start=True, stop=False)
            nc.tensor.matmul(kp[:], ksum[:], Tb[:, 1, :], start=False, stop=True)
            nc.scalar.activation(
                at[0:RES, 0].rearrange("p b x -> p (b x)"), kp[:],
                AF.Tanh, bias=zbias[0:RES, :], scale=float(slope) * post1,
            )
            # tail is just trigger -> transfer -> completion sem: the 625ns
            # HWDGE + 650ns DGE-DMA delay of the plain DMA path vanish.
            # The placeholder osem wait is repointed post-compile at the Act
            # engine-tick sem (activations only carry ONE hw sem update, so
            # a then_inc on the final act is rejected by codegen).  The SP
            # sem_inc only satisfies the scheduler's deadlock check.
            nc.sync.sem_inc(osem, 1)
            nc.gpsimd.trigger_dma(count=None, queue_num=0)._wait_ge(osem, 1)

    nc.compile()
    # A SWDGE prep's single hardware completion-sem slot must hold the
    # DMASW lane sem Tile pointed the data consumers at; the sem= handle we
    # passed only placates the API.  Repoint update[0] accordingly.
    # (After compile: the sem waits materialize during compile().)
    _patch_prep_sems(nc)
    return nc


def _patch_prep_sems(nc):
    from concourse.tile_sem_assignment import PROC_NAME_TO_IDX

    idx_to_lane = {v: k for k, v in PROC_NAME_TO_IDX.items()
                   if k.startswith("DMASW")}
    fn = nc.m.functions[0]
    lane_ids = {}
    for b in fn.blocks:
        for i in b.instructions:
            si = i.sync_info
            if si is None:
                continue
            for w in list(si.on_wait) + list(si.on_update):
                nm = str(w.ant_name or "")
                if nm.startswith("DMASW"):
                    lane_ids[nm.split("_")[0]] = (w.id, w.ant_name)
    kv_lane_sem = None
    for b in fn.blocks:
        for i in b.instructions:
            if type(i).__name__ in ("InstDMAGatherAnt", "InstKVWritebackAnt"):
                lane = idx_to_lane[i.bass_scheduled_proc]
                sid, sname = lane_ids[lane]
                si = i.sync_info
                u0 = list(si.on_update)[0]
                assert str(u0.ant_name) == "out_dma", u0
                u0.id = sid
                u0.ant_name = sname
                if type(i).__name__ == "InstKVWritebackAnt":
                    kv_lane_sem = str(sname)
    # Tile guards the final-act write of `at` behind the out-DMA completion
    # (WAR vs the early desc-prep's deferred read).  That read really happens
    # at the trigger, strictly after the act, so the guard is a false cycle:
    # act -> trigger -> DMA -> act.  Defuse the Activation-side guard.
    assert kv_lane_sem is not None
    for b in fn.blocks:
        for i in b.instructions:
            if (type(i).__name__ == "InstEventSemaphore"
                    and i.engine == mybir.EngineType.Activation):
                si = i.sync_info
                for w in (si.on_wait if si else []):
                    if str(w.ant_name) == kv_lane_sem:
                        w.wait_value = 0
    # Repoint the out-trigger's placeholder osem wait at the Act engine-tick
    # sem value reached by the FINAL activation (activations carry only one
    # hw sem update, so the act itself cannot inc a user sem).
    act_sem = None   # (id, name) of the Act engine tick sem
    act_ticks = 0
    ticks_at_last_act = 0
    for b in fn.blocks:
        for i in b.instructions:
            si = i.sync_info
            if si is None:
                continue
            for u in si.on_update:
                nm = str(u.ant_name or "")
                if nm.startswith("Activation_"):
                    act_sem = (u.id, u.ant_name)
                    act_ticks += int(u.update_value or 1)
                    if type(i).__name__ == "InstActivation":
                        ticks_at_last_act = act_ticks
    assert act_sem is not None and ticks_at_last_act > 0
    for b in fn.blocks:
        for i in b.instructions:
            if type(i).__name__ == "InstTriggerDma":
                si = i.sync_info
                for w in (si.on_wait if si else []):
                    if str(w.ant_name) == "out_ready":
                        w.id = act_sem[0]
                        w.ant_name = act_sem[1]
                        w.wait_value = ticks_at_last_act


_CACHE = {}


def _get_program(sigma, slope_strk, slope):
    key = (float(sigma), float(slope_strk), float(slope))
    if key not in _CACHE:
        _CACHE[key] = _build_program(*key)
    return _CACHE[key]


def _basis(steps):
    t = np.linspace(0.0, 1.0, steps, dtype=np.float64)[:, None]
    i = np.arange(PTS, dtype=np.float64)[None, :]
    binom = np.array([comb(PTS - 1, j) for j in range(PTS)],
                     dtype=np.float64)[None, :]
    return binom * (t ** i) * ((1.0 - t) ** (PTS - 1 - i))      # [steps, 5]


def _host_consts():
    grid = np.linspace(0.0, 1.0, RES, dtype=np.float64)
    # gmat rows: [G0; G0; G1; 1; 1; g2h; g2l] (fp16)
    G0 = (-2.0 * grid).astype(np.float16)
    G1 = (-2.0 * grid - G0.astype(np.float64)).astype(np.float16)
    g2 = grid ** 2
    g2h = g2.astype(np.float16)
    g2l = (g2 - g2h.astype(np.float64)).astype(np.float16)
    ones = np.ones(RES, np.float16)
    gmat = np.stack([G0, G0, G1, ones, ones, g2h, g2l])          # [7, 28]
    # deltaH[(h'*8+g'), (h, g, x)] = delta(h,h')*delta(g,g')
    deltaH = np.kron(np.eye(NH * BL, dtype=np.float32),
                     np.ones((1, RES), np.float32))              # [16, 448]
    # ksum64[(j2*32+y), y'] = delta(y,y') for y<28, zero pad rows
    ksum = np.zeros((SP, RES), dtype=np.float32)
    for j2 in range(2):
        ksum[j2 * 32 : j2 * 32 + RES] = np.eye(RES, dtype=np.float32)
    return gmat, deltaH, ksum


def _host_inputs(z_pres, z_what, z_where, sigma):
    """Per-core input dicts: tiny curve linear algebra done host-side."""
    import ml_dtypes

    gmat, deltaH, ksum = _host_consts()
    Bm = _basis(T)                                            # [T, 5]
    inv = 1.0 / (2.0 * sigma * sigma)
    A = (STEPS_REF - 1) / (T - 1)
    w = np.full(T, A, dtype=np.float64)
    w[0] = w[-1] = (1.0 + A) / 2.0
    # fold the per-sample weight into the c^2 rows: per coord the Exp arg
    # gains +0.5*ln(w_t), i.e. c2' = c^2 - 0.5*ln(w_t)/inv
    c2_off = 0.5 * np.log(w) / inv                            # [T]

    s = z_where[..., 0].astype(np.float64)
    shift = z_where[..., 1:3].astype(np.float64)
    pts = z_what.astype(np.float64) * s[..., None, None] + shift[..., None, :]
    curve = np.einsum("tp,bkpd->bktd", Bm, pts)               # [64,4,T,2]

    ksum_bf = ksum.astype(ml_dtypes.bfloat16)
    in_maps = []
    for cidx in range(NCORES):
        sl = slice(cidx * BL, (cidx + 1) * BL)
        cv = curve[sl].reshape(NPAIR, T, 2)                   # [32,T,2]
        lhsy = np.zeros((RPP, LYC), dtype=np.float16)
        lhsx = np.zeros((RPP, LXC), dtype=np.float16)
        for p in range(NPAIR):
            for coord in range(2):
                cc = cv[p, :, coord]                          # [T] f64
                c0 = cc.astype(np.float16)
                c1 = (cc - c0.astype(np.float64)).astype(np.float16)
                c2 = cc ** 2 - c2_off
                c2h = c2.astype(np.float16)
                c2l = (c2 - c2h.astype(np.float64)).astype(np.float16)
                blk = np.stack([c0, c1, c0, c2h, c2l,
                                np.ones(T, np.float16), np.ones(T, np.float16)])
                dst = lhsy if coord == 1 else lhsx
                dst[:, p * T : (p + 1) * T] = blk
        lhsy[0:RPP, C_G : C_G + RES] = gmat
        # deltah payload: [16, 448] selector + ezi [16,2] f32 in 4 bf16 cols
        zp = z_pres[sl].astype(np.float64)                    # [8,4]
        ezi = np.zeros((NH * BL, 2), dtype=np.float32)
        for h in range(NH):
            for g in range(BL):
                for j2 in range(2):
                    ezi[h * BL + g, j2] = EPS / max(zp[g, 2 * h + j2], 1e-30)
        dh = np.zeros((NH * BL, DHC), dtype=ml_dtypes.bfloat16)
        dh[:, 0 : NH * BL * RES] = deltaH.astype(ml_dtypes.bfloat16)
        dh[:, NH * BL * RES : DHC] = ezi.view(np.uint16).view(ml_dtypes.bfloat16)
        m = {"ksum": ksum_bf, "deltah": dh}
        m["lhsy"] = lhsy
        m["lhsx"] = lhsx
        in_maps.append(m)
    return in_maps


def kernel(z_pres, z_what, z_where, sigma, slope_strk, slope):
    z_pres = np.asarray(z_pres, np.float32)
    z_what = np.asarray(z_what, np.float32)
    z_where = np.asarray(z_where, np.float32)
    nc = _get_program(float(sigma), float(slope_strk), float(slope))
    in_maps = _host_inputs(z_pres, z_what, z_where, float(sigma))
    res = run_bass_kernel_spmd(nc, in_maps, core_ids=list(range(NCORES)))
    out = np.concatenate([np.asarray(r["out"], np.float32)[:, 0:RES, 0, :]
                          for r in res.results], axis=0)            # [64,28,28]
    out = out * np.float32(1.0 / tanh(float(slope)))   # post-scale on host
    return out[:, None].astype(np.float32)
